# revision 43
# baseline (speedup 1.0000x reference)
"""Grouped-Query Attention kernel for Trainium2 (8 NeuronCores, SPMD).

Problem: x [4, 4096, 512] fp32, per-group Dense Q/K/V (G=4 groups of 128
features), full softmax attention within each (batch, group) pair, output
re-concatenated to [4, 4096, 512].

Sharding: B*G = 16 fully independent attention problems -> 2 per core.
Per core, per pair, everything stays on-chip (SBUF 24MB).

Pipeline (v3):
  - x loaded fp32 (chunked DMA), cast bf16 (ScalarE), PE-transposed to
    xgT [d, t]
  - Q^T/K^T = W^T xgT (bias folded) bf16; V natural [t, e] via
    xgT-chunk-stationary matmuls, quantized fp8 e4m3 (+bv folded in)
  - scores computed TRANSPOSED per 128-key chunk: S^T[ts, tq] = K_c Q^T
    (bf16 matmuls)
  - exp with scale=1/sqrt(gs) and bias=-3 folded in (the e^-3 shift
    cancels in softmax; keeps exp < 240 = trn-e4m3 max), written
    DIRECTLY as fp8 e4m3 into chunk-pair tiles [128, 2, 1024].
    5/8 of chunks use ScalarE ACT Exp; 3/8 use a Schraudolph exp2
    bit-trick on DVE (fp32->u8 rint+saturate == e4m3 bits), balancing
    the two engines under the PE.
  - P@V and the ones-denominator matmul run in fp8 DoubleRow perf mode
    (K=256: two key chunks per pass -> half the streamed columns)
  - epilogue (spread across later iterations, software-pipelined across
    macro boundaries): PSUM evacuated via fast bf16 copies, denominator
    columns extracted with one-hot matmuls, tiny reciprocal [128,8],
    PE-transpose back, per-partition normalize on DVE, DMA out fp32
  - the next pair's prologue is pumped through the previous pair's
    macro-loop PE slack; HAM warm-up dummies cover the initial DMA wait
Compute dtype bf16 for scores, fp8 for probabilities/V (fp32 PSUM acc).
"""

import os
import sys

sys.path.insert(0, "/opt/trn_rl_repo")

import numpy as np

import concourse.bass as bass
import concourse.mybir as mybir
import concourse.tile as tile
from concourse.masks import make_identity

B, T, F, G = 4, 4096, 512, 4
GS = F // G  # 128
N_CORES = 8
PAIRS_PER_CORE = (B * G) // N_CORES  # 2
TQ_MACRO = 1024  # query tile width per softmax/psum round
N_MACROS = T // TQ_MACRO  # 4
N_CHUNKS = T // 128  # 32 key/time chunks
N_CPAIRS = N_CHUNKS // 2  # 16 chunk pairs (DoubleRow K=256)
INV_SCALE = float(1.0 / (np.sqrt(np.float32(GS)) + 1e-9))
EXP_SHIFT = -3.0  # exp(s - 3): cancels in softmax, keeps max < 240
# Schraudolph exp directly in fp8-e4m3 bit space, computed on DVE:
#   u8 = rint(SCH_A * s_raw + SCH_B); bitcast u8 -> e4m3 ~= exp(s*INV_SCALE-3)
# (DVE fp32->u8 conversion is rint+saturate; negatives flush to 0.)
LOG2E = float(np.log2(np.e))
SCH_A = 8.0 * LOG2E * INV_SCALE
SCH_B = 8.0 * (7.0 + EXP_SHIFT * LOG2E)

FP32 = mybir.dt.float32
BF16 = mybir.dt.bfloat16
FP8 = mybir.dt.float8e4
U8 = mybir.dt.uint8
DR = mybir.MatmulPerfMode.DoubleRow

_NC_CACHE = None
_LAST_IN_MAPS = None


def _split_multi_waits(nc):
    """Walrus codegen rejects instructions carrying more than one semaphore
    wait on several instruction structs (DMA DIRECT2D, tensor_scalar, LDW).
    Hoist all-but-the-last wait of any multi-wait instruction onto same-engine
    NoOps inserted immediately before it: the sequencer executes them in
    order, so the gating semantics are identical."""
    n_split = 0
    for func in nc.m.functions:
        for block in func.blocks:
            new = []
            for inst in block.instructions:
                si = inst.sync_info
                waits = list(si.on_wait) if (si is not None and si.on_wait) else []
                if len(waits) > 1:
                    for w in waits[:-1]:
                        nop = mybir.InstNoOp(
                            name=nc.get_next_instruction_name(), ins=[], outs=[]
                        )
                        nop.engine = inst.engine
                        nop.sync_info = mybir.SyncInfo(on_wait=[w], on_update=[])
                        new.append(nop)
                        n_split += 1
                    inst.sync_info = mybir.SyncInfo(
                        on_wait=[waits[-1]],
                        on_update=list(si.on_update) if si.on_update else [],
                    )
                new.append(inst)
            block.instructions = new
    return n_split


def build_nc():
    nc = bass.Bass()

    ins = []
    outs = []
    for i in range(PAIRS_PER_CORE):
        ins.append(
            dict(
                x=nc.declare_dram_parameter(f"x{i}", [T, GS], FP32, isOutput=False),
                wq=nc.declare_dram_parameter(f"wq{i}", [GS, GS], FP32, isOutput=False),
                wk=nc.declare_dram_parameter(f"wk{i}", [GS, GS], FP32, isOutput=False),
                wv=nc.declare_dram_parameter(f"wv{i}", [GS, GS], FP32, isOutput=False),
                bq=nc.declare_dram_parameter(f"bq{i}", [1, GS], FP32, isOutput=False),
                bk=nc.declare_dram_parameter(f"bk{i}", [1, GS], FP32, isOutput=False),
                bv=nc.declare_dram_parameter(f"bv{i}", [1, GS], FP32, isOutput=False),
            )
        )
        outs.append(nc.declare_dram_parameter(f"y{i}", [T, GS], FP32, isOutput=True))

    with tile.TileContext(nc) as tc:
        with (
            tc.tile_pool(name="consts", bufs=1) as consts,
            tc.tile_pool(name="bigsb", bufs=2) as bigsb,  # per-pair persistent
            tc.tile_pool(name="pt", bufs=5) as ptpool,  # exp'd prob chunk-pairs
            tc.tile_pool(name="epi", bufs=3) as epi,  # epilogue sbuf tiles
            tc.tile_pool(name="ps_s", bufs=2, space="PSUM") as ps_s,  # scores
            tc.tile_pool(name="ps_o", bufs=1, space="PSUM") as ps_o,  # out^T
            tc.tile_pool(name="ps_d", bufs=1, space="PSUM") as ps_d,  # denom
        ):
            ident_bf = consts.tile([128, 128], BF16)
            make_identity(nc, ident_bf)
            ones2_f8 = consts.tile([128, 2, 128], FP8)
            nc.vector.memset(ones2_f8, 1.0)
            exp_bias = consts.tile([128, 1], FP32)
            nc.vector.memset(exp_bias, EXP_SHIFT)

            # HAM warm-up: ~60 dependency-free matmuls while the first x
            # DMA chunks land. Sustained PE activity for >3.4us releases the
            # clock gate (1.2 -> 2.4 GHz) before the real prologue starts.
            for _ in range(60):
                scr = ps_s.tile([128, 128], BF16, tag="sc", name="warm")
                nc.tensor.transpose(scr, ident_bf, ident_bf)

            # ---- prefetch: x + weights for BOTH pairs up-front ----
            xg_fs = []
            w_sb = []
            for i in range(PAIRS_PER_CORE):
                p = ins[i]
                xg_f = bigsb.tile([128, N_CHUNKS, 128], FP32, tag="xg_f")
                for d in range(8):
                    nc.sync.dma_start(
                        out=xg_f[:, d * 4 : (d + 1) * 4, :],
                        in_=p["x"][d * 512 : (d + 1) * 512, :].rearrange(
                            "(c p) d -> p c d", p=128
                        ),
                    )
                xg_fs.append(xg_f)
                wd = {}
                for nm in ("wq", "wk", "wv"):
                    wf = epi.tile([128, 128], FP32, tag=f"wf{nm}{i}")
                    nc.gpsimd.dma_start(out=wf, in_=p[nm][:, :])
                    wb = consts.tile([128, 128], BF16, tag=f"{nm}{i}")
                    nc.vector.tensor_copy(wb, wf)
                    wd[nm] = wb
                for nm in ("bq", "bk"):
                    bc = consts.tile([128, 1], FP32, tag=f"{nm}{i}")
                    nc.gpsimd.dma_start(
                        out=bc, in_=p[nm][:, :].rearrange("o d -> d o")
                    )
                    wd[nm] = bc
                # bv broadcast across partitions [128, 128] (folded into V)
                bvb = consts.tile([128, 128], FP32, tag=f"bvb{i}")
                _bv = p["bv"][:, :]
                nc.gpsimd.dma_start(
                    out=bvb,
                    in_=bass.AP(tensor=_bv.tensor, offset=_bv.offset,
                                ap=[[0, 128]] + list(_bv.ap[1:])),
                )
                wd["bvb"] = bvb
                w_sb.append(wd)

            # per-pair persistent tiles (bigsb bufs=2 rotates by tag)
            pair_tiles = []
            for i in range(PAIRS_PER_CORE):
                pair_tiles.append(dict(
                    xg_b=bigsb.tile([128, N_CHUNKS, 128], BF16, tag="xg_b",
                                    name=f"xg_b{i}"),
                    xgT=bigsb.tile([128, T], BF16, tag="xgT", name=f"xgT{i}"),
                    qt=bigsb.tile([128, T], BF16, tag="qt", name=f"qt{i}"),
                    kt=bigsb.tile([128, T], BF16, tag="kt", name=f"kt{i}"),
                    v8=bigsb.tile([128, N_CHUNKS, 128], FP8, tag="v8",
                                  name=f"v8_{i}"),
                ))

            def _qt_slice(i, jsl):
                """Emit Q^T (jsl'th TQ_MACRO slice) for pair i."""
                w_bf, pt = w_sb[i], pair_tiles[i]
                psq = ps_s.tile([128, TQ_MACRO], FP32, tag="sc", name="psq")
                for h in range(TQ_MACRO // 512):
                    sl = slice(h * 512, (h + 1) * 512)
                    tsl = slice(
                        jsl * TQ_MACRO + h * 512, jsl * TQ_MACRO + (h + 1) * 512
                    )
                    nc.tensor.matmul(
                        psq[:, sl], w_bf["wq"], pt["xgT"][:, tsl],
                        start=True, stop=True,
                    )
                dsl = slice(jsl * TQ_MACRO, (jsl + 1) * TQ_MACRO)
                nc.vector.tensor_scalar_add(pt["qt"][:, dsl], psq, w_bf["bq"])

            def _cast_group(i, d):
                # on ScalarE: keeps the busy DVE off the prologue path
                pt = pair_tiles[i]
                nc.scalar.copy(
                    pt["xg_b"][:, d * 4 : (d + 1) * 4, :],
                    xg_fs[i][:, d * 4 : (d + 1) * 4, :],
                )

            def _transposes(i, c0, c1):
                pt = pair_tiles[i]
                for c in range(c0, c1):
                    pst = ps_s.tile([128, 128], BF16, tag="sc")
                    nc.tensor.transpose(pst, pt["xg_b"][:, c, :], ident_bf)
                    nc.scalar.copy(
                        pt["xgT"][:, c * 128 : (c + 1) * 128], pst
                    )

            def _kt_slice(i, jsl):
                w_bf, pt = w_sb[i], pair_tiles[i]
                psq = ps_s.tile([128, TQ_MACRO], FP32, tag="sc", name="psk")
                for h in range(TQ_MACRO // 512):
                    sl = slice(h * 512, (h + 1) * 512)
                    tsl = slice(
                        jsl * TQ_MACRO + h * 512, jsl * TQ_MACRO + (h + 1) * 512
                    )
                    nc.tensor.matmul(
                        psq[:, sl], w_bf["wk"], pt["xgT"][:, tsl],
                        start=True, stop=True,
                    )
                dsl = slice(jsl * TQ_MACRO, (jsl + 1) * TQ_MACRO)
                nc.vector.tensor_scalar_add(pt["kt"][:, dsl], psq, w_bf["bk"])

            def _v_chunks(i, c0, c1):
                w_bf, pt = w_sb[i], pair_tiles[i]
                for c in range(c0, c1):
                    psv = ps_s.tile([128, 128], FP32, tag="sc")
                    nc.tensor.matmul(
                        psv, pt["xgT"][:, c * 128 : (c + 1) * 128], w_bf["wv"],
                        start=True, stop=True,
                    )
                    nc.vector.tensor_add(pt["v8"][:, c, :], psv, w_bf["bvb"])

            def prologue_a(i):
                """Minimum pair i's first macro iterations need up-front:
                x cast (first half), xgT transposes 0-15, K^T slices 0-1,
                Q^T slice 0."""
                for d in range(4):
                    _cast_group(i, d)
                for c0 in range(0, 16, 2):
                    _transposes(i, c0, c0 + 2)
                    yield
                _kt_slice(i, 0)
                yield
                _kt_slice(i, 1)
                yield
                _qt_slice(i, 0)
                yield

            def prologue_b(i):
                """Pumped one quantum per iteration of pair i's first macro:
                remaining casts/transposes/K^T/Q^T slices + V chunks, paced
                so every consumer finds its data emitted in time."""
                for q in range(N_CPAIRS):
                    if q < 2:
                        _cast_group(i, 4 + 2 * q)
                        _cast_group(i, 5 + 2 * q)
                    if q < 4:
                        _transposes(i, 16 + 4 * q, 20 + 4 * q)
                    elif q == 4:
                        _kt_slice(i, 2)
                    elif q == 5:
                        _kt_slice(i, 3)
                    elif q in (6, 7, 8):
                        _qt_slice(i, q - 5)
                    _v_chunks(i, 2 * q, 2 * q + 2)
                    yield

            pending_epi = []
            pv_queue = []
            pro_a = [prologue_a(i) for i in range(PAIRS_PER_CORE)]
            pro_b = [prologue_b(i) for i in range(PAIRS_PER_CORE)]

            def pump_gen(gens, gen_idx, n=1):
                if gen_idx >= PAIRS_PER_CORE:
                    return False
                g = gens[gen_idx]
                if g is None:
                    return False
                try:
                    for _ in range(n):
                        next(g)
                    return True
                except StopIteration:
                    gens[gen_idx] = None
                    return False

            pump_gen(pro_a, 0, 10**9)  # pair 0 stage A can't hide

            for i in range(PAIRS_PER_CORE):
                pt_i = pair_tiles[i]
                qt, kt, v8 = pt_i["qt"], pt_i["kt"], pt_i["v8"]
                pump_gen(pro_a, i, 10**9)  # finish stage A remainder

                # ---------------- attention macros ----------------
                # Cross-macro software pipeline: the fp8 PV+den matmuls for
                # chunk-pair j are emitted 2 iterations later (possibly in the
                # NEXT macro), so the PE never waits on the trailing ACTs and
                # HAM stays warm. The last PV unit of a macro also frees PSUM
                # via cheap bf16 copies and queues the PE transpose epilogue.
                for m in range(N_MACROS):
                    tq0 = m * TQ_MACRO
                    ps_out = ps_o.tile([128, TQ_MACRO], FP32)
                    ps_den = ps_d.tile([128, TQ_MACRO], FP32)

                    def _mk_pv(pj, ppt, ps_out=ps_out, ps_den=ps_den,
                               tq0=tq0, v8=v8, out_dram=outs[i]):
                        def _pv():
                            first, last = pj == 0, pj == N_CPAIRS - 1
                            for h in range(TQ_MACRO // 512):
                                sl = slice(h * 512, (h + 1) * 512)
                                nc.tensor.matmul(
                                    ps_out[:, sl], v8[:, 2 * pj : 2 * pj + 2, :],
                                    ppt[:, :, sl], start=first, stop=last,
                                    perf_mode=DR,
                                )
                                nc.tensor.matmul(
                                    ps_den[:, sl], ones2_f8, ppt[:, :, sl],
                                    start=first, stop=last, perf_mode=DR,
                                )
                            if not last:
                                return
                            # macro complete: evacuate PSUM via fast bf16
                            # copies (frees ps_out/ps_den for the next macro),
                            # defer transposes + normalization in small pieces.
                            ot_b = epi.tile([128, TQ_MACRO], BF16, tag="ot")
                            nc.vector.tensor_copy(ot_b, ps_out)
                            dt_b = epi.tile([128, TQ_MACRO], BF16, tag="dt")
                            nc.vector.tensor_copy(dt_b, ps_den)
                            rcols = epi.tile(
                                [128, TQ_MACRO // 128], FP32, tag="rcols"
                            )
                            onat = epi.tile(
                                [128, TQ_MACRO // 128, 128], FP32, tag="onat"
                            )

                            def _epi_a():
                                # denominator columns via one-hot matmuls:
                                # dcols[t, j] = dt_b[0, j*128+t]
                                dcols = ps_s.tile(
                                    [128, TQ_MACRO // 128], FP32, tag="sc"
                                )
                                for jj in range(TQ_MACRO // 128):
                                    nc.tensor.matmul(
                                        dcols[:, jj : jj + 1],
                                        dt_b[:, jj * 128 : (jj + 1) * 128],
                                        ident_bf[:, 0:1],
                                        start=True, stop=True,
                                    )
                                nc.vector.reciprocal(rcols, dcols)

                            def _epi_b(j0, j1):
                                def _f():
                                    for jj in range(j0, j1):
                                        tp = ps_s.tile([128, 128], BF16, tag="sc")
                                        nc.tensor.transpose(
                                            tp,
                                            ot_b[:, jj * 128 : (jj + 1) * 128],
                                            ident_bf,
                                        )
                                        nc.vector.tensor_scalar_mul(
                                            onat[:, jj, :], tp, rcols[:, jj : jj + 1]
                                        )
                                        if jj % 4 == 3:
                                            hh = jj // 4
                                            nc.gpsimd.dma_start(
                                                out=out_dram[
                                                    tq0 + hh * 512 :
                                                    tq0 + (hh + 1) * 512,
                                                    :,
                                                ].rearrange(
                                                    "(c p) d -> p c d", p=128
                                                ),
                                                in_=onat[:, hh * 4 : (hh + 1) * 4, :],
                                            )
                                return _f

                            pending_epi.extend(
                                [_epi_a, _epi_b(0, 3), _epi_b(3, 6), _epi_b(6, 8)]
                            )

                        return _pv

                    last_macro = i == PAIRS_PER_CORE - 1 and m == N_MACROS - 1
                    for j in range(N_CPAIRS):
                        # PE: fp8 PV + den, 2 chunk-pairs behind, emitted
                        # FIRST: this work is always ready (2-iteration
                        # slack), so the PE chews on it while the previous
                        # exp finishes instead of head-blocking on the
                        # scores' WAR. (Drained at the very end -> short tail)
                        depth = 0 if last_macro and j >= N_CPAIRS - 3 else 1
                        while len(pv_queue) > depth:
                            pv_queue.pop(0)()
                        # scores for chunks 2j, 2j+1 (bf16), exp emitted
                        # per-chunk so ScalarE/DVE start as early as possible.
                        # 3 of every 8 chunks run the Schraudolph bit-trick
                        # on the DVE instead of ScalarE, balancing engines.
                        pt2 = ptpool.tile([128, 2, TQ_MACRO], FP8)
                        for ci in range(2):
                            c = 2 * j + ci
                            ksl = kt[:, c * 128 : (c + 1) * 128]
                            ps_sc = ps_s.tile([128, TQ_MACRO], FP32, tag="sc")
                            for h in range(TQ_MACRO // 512):
                                sl = slice(h * 512, (h + 1) * 512)
                                qsl = slice(tq0 + h * 512, tq0 + (h + 1) * 512)
                                nc.tensor.matmul(
                                    ps_sc[:, sl], ksl, qt[:, qsl],
                                    start=True, stop=True,
                                )
                            if c % 8 in (1, 3, 7):
                                nc.vector.tensor_scalar(
                                    pt2[:, ci, :].bitcast(U8), ps_sc,
                                    SCH_A, SCH_B,
                                    op0=mybir.AluOpType.mult,
                                    op1=mybir.AluOpType.add,
                                )
                            else:
                                nc.scalar.activation(
                                    pt2[:, ci, :], ps_sc,
                                    mybir.ActivationFunctionType.Exp,
                                    scale=INV_SCALE, bias=exp_bias,
                                )
                        pv_queue.append(_mk_pv(j, pt2))
                        if j >= 2 and pending_epi:
                            pending_epi.pop(0)()
                            if last_macro and pending_epi:
                                pending_epi.pop(0)()
                        # own stage-B prologue / next pair's prologue ride in
                        # the tail of the iteration's PE slack
                        if m == 0:
                            pump_gen(pro_b, i)
                        elif not pump_gen(pro_a, i + 1):
                            pump_gen(pro_b, i + 1)
            while pv_queue:
                pv_queue.pop(0)()
            for f in pending_epi:
                f()
    _split_multi_waits(nc)
    return nc


def _get_nc():
    global _NC_CACHE
    if _NC_CACHE is None:
        _NC_CACHE = build_nc()
    return _NC_CACHE


def kernel(**inputs: np.ndarray) -> np.ndarray:
    x = np.ascontiguousarray(inputs["x"], dtype=np.float32)
    Wq = np.asarray(inputs["Wq"], dtype=np.float32)
    Wk = np.asarray(inputs["Wk"], dtype=np.float32)
    Wv = np.asarray(inputs["Wv"], dtype=np.float32)
    bq = np.asarray(inputs["bq"], dtype=np.float32)
    bk = np.asarray(inputs["bk"], dtype=np.float32)
    bv = np.asarray(inputs["bv"], dtype=np.float32)

    nc = _get_nc()

    in_maps = []
    for core in range(N_CORES):
        m = {}
        for i in range(PAIRS_PER_CORE):
            pair = core * PAIRS_PER_CORE + i
            b, g = pair // G, pair % G
            sl = slice(g * GS, (g + 1) * GS)
            m[f"x{i}"] = np.ascontiguousarray(x[b, :, sl])
            m[f"wq{i}"] = np.ascontiguousarray(Wq[g])
            m[f"wk{i}"] = np.ascontiguousarray(Wk[g])
            m[f"wv{i}"] = np.ascontiguousarray(Wv[g])
            m[f"bq{i}"] = np.ascontiguousarray(bq[g].reshape(1, GS))
            m[f"bk{i}"] = np.ascontiguousarray(bk[g].reshape(1, GS))
            m[f"bv{i}"] = np.ascontiguousarray(bv[g].reshape(1, GS))
        in_maps.append(m)

    global _LAST_IN_MAPS
    _LAST_IN_MAPS = in_maps

    from concourse.bass_utils import run_bass_kernel_spmd

    res = run_bass_kernel_spmd(nc, in_maps, list(range(N_CORES)))

    y = np.empty((B, T, F), dtype=np.float32)
    for core in range(N_CORES):
        for i in range(PAIRS_PER_CORE):
            pair = core * PAIRS_PER_CORE + i
            b, g = pair // G, pair % G
            y[b, :, g * GS : (g + 1) * GS] = res.results[core][f"y{i}"]
    return y
